# revision 11
# baseline (speedup 1.0000x reference)
"""Trainium2 Bass kernel for nn_Attention_51307679318359.

Multi-head attention (B=2, S=2048, D=2048, H=16, HD=128) with RoPE and an
additive mask, sharded over 8 NeuronCores as (batch x head-group): each core
computes 1 batch and 4 heads (512 channels), producing a partial output that
the host sums over head-groups.

v2 design (bf16 end-to-end, fully SBUF-resident, software-pipelined):
  - All matmul operands in bf16 (PSUM accumulation stays f32): halves DMA
    and SBUF, and LDWEIGHTS runs with FWL (fp32 weight loads can't).
  - q/k/v for all 4 heads stay SBUF-resident between projection and
    attention -- no DRAM spill round trip.
  - Work is organized per sq-chunk jq (512 columns): A(jq) = QKV projection
    for chunk jq, B(jq) = attention blocks for all heads at chunk jq,
    C(jq) = output projection rows for chunk jq.  Emission interleaves the
    ACT-bound B(jq-1) blocks with PE-dense fillers (A(jq) groups, C(jq-2)
    groups) so the tensor engine never waits on the Exp chain.
  - Softmax denominator: probs blocks accumulate on DVE into an fp16 tile
    (4x DVE mode); the PE only runs one ones-vector matmul per (h, jq)
    instead of one per block.
  - Causal structure: fully-masked 128x128 sub-blocks are skipped by
    column-slicing each (i, jq) block; only mixed sub-blocks get a
    multiplicative exp(mask) pattern (a single [128,128] triangle for the
    causal mask).
"""

import math

import numpy as np

import concourse.bass as bass
import concourse.mybir as mybir
import concourse.tile as tile
from concourse import bacc
from concourse import bass_utils

try:
    import ml_dtypes
    _BF16 = ml_dtypes.bfloat16
    _FP16 = np.float16
except Exception:  # pragma: no cover
    _BF16 = None
    _FP16 = np.float16

F32 = mybir.dt.float32
BF16 = mybir.dt.bfloat16
F16 = mybir.dt.float16
ADD = mybir.AluOpType.add
MULT = mybir.AluOpType.mult
EXP = mybir.ActivationFunctionType.Exp
COPY = mybir.ActivationFunctionType.Copy

B, S, D = 2, 2048, 2048
H, HD = 16, 128
NCORES = 8
GROUPS = NCORES // B          # 4 head-groups
HPG = H // GROUPS             # 4 heads per group
C = HPG * HD                  # 512 per-core channels
P = 128
CH = 512                      # s-chunk width (both projection and attention)
NJ = S // CH                  # 4 chunks
NKT = D // P                  # 16 contraction tiles
SCALE = 1.0 / math.sqrt(HD)
NEG_THRESH = -1e8             # "masked out" threshold
SUB = CH // P                 # 4 sub-columns of 128 per chunk

_PROGRAM_CACHE = {}


def _pre_w(wT):
    """(d, c) row-major -> (128, d//128, c) partition-major contiguous."""
    d, c = wT.shape
    return np.ascontiguousarray(wT.reshape(d // P, P, c).transpose(1, 0, 2))


def _classify_mask(mask):
    """Classify the mask at 128x128 sub-block granularity.

    Returns (blocks, pats):
      blocks[jq] = list of (i, c0, w, [(c_rel, pat_idx), ...]) -- live
        sk-tile i contributes columns [c0*128, (c0+w)*128) of chunk jq;
        listed sub-columns get a multiplicative exp(mask) pattern.
      pats: (npat, 128, 128) f32 of exp(mask sub-block).
    The first block of each jq list covers the full live column range so
    the PSUM accumulation group can start with a full-width write.
    """
    maskT = np.ascontiguousarray(mask.T)
    n_i = mask.shape[0] // P
    patterns = []
    pat_idx = {}

    def add_pattern(blk):
        key = blk.tobytes()
        if key not in pat_idx:
            pat_idx[key] = len(patterns)
            with np.errstate(over='ignore'):
                patterns.append(np.exp(blk.astype(np.float64)).astype(np.float32))
        return pat_idx[key]

    def classify(jq, i, dense):
        sub_cls = []
        for c in range(SUB):
            blk = maskT[i * P:(i + 1) * P,
                        jq * CH + c * P: jq * CH + (c + 1) * P]
            if not dense and np.all(blk <= NEG_THRESH):
                sub_cls.append(('masked', None))
            elif np.all(blk == 0.0):
                sub_cls.append(('clear', None))
            else:
                sub_cls.append(('pat', blk))
        live = [c for c, (k, _) in enumerate(sub_cls) if k != 'masked']
        if not live:
            return None
        c0, c1 = live[0], live[-1]
        pats = []
        for c in range(c0, c1 + 1):
            kind, blk = sub_cls[c]
            if kind == 'masked':
                # non-contiguous live range: zero out via an all-zero pattern
                blk = np.full((P, P), -np.inf, np.float32)
                kind = 'pat'
            if kind == 'pat':
                pats.append((c - c0, add_pattern(blk)))
        return (i, c0, c1 - c0 + 1, pats)

    def build(dense):
        blocks = {}
        ok = True
        for jq in range(NJ):
            lst = []
            for i in range(n_i):
                r = classify(jq, i, dense)
                if r is not None:
                    lst.append(r)
            # first block must cover the full live column range of the chunk
            if not lst:
                ok = False
                break
            lo = min(r[1] for r in lst)
            hi = max(r[1] + r[2] for r in lst)
            full = [k for k, r in enumerate(lst) if r[1] == lo and r[1] + r[2] == hi]
            if not full:
                ok = False
                break
            lst.insert(0, lst.pop(full[0]))
            # every live column must be covered (else softmax denom is 0)
            cov = np.zeros(SUB, bool)
            for (_, c0, w, _) in lst:
                cov[c0:c0 + w] = True
            if not cov[lo:hi].all():
                ok = False
                break
            blocks[jq] = lst
        return blocks if ok else None

    blocks = build(dense=False)
    if blocks is None:
        patterns.clear()
        pat_idx.clear()
        blocks = build(dense=True)
        assert blocks is not None
    pats = np.stack(patterns, 0) if patterns else np.zeros((1, P, P), np.float32)
    return blocks, pats


def _blocks_key(blocks, npat):
    return (npat, tuple(
        (jq, tuple((i, c0, w, tuple(ps)) for (i, c0, w, ps) in lst))
        for jq, lst in sorted(blocks.items())))


def _merge(primary, fillers):
    """Interleave fillers evenly between primary units (fillers lead)."""
    out = []
    i = j = 0
    np_, nf = len(primary), len(fillers)
    while i < np_ or j < nf:
        if j < nf and (i >= np_ or j * np_ <= i * nf):
            out.append(fillers[j]); j += 1
        else:
            out.append(primary[i]); i += 1
    return out


def _rr(a, b):
    """Round-robin two lists proportionally."""
    return _merge(a, b) if len(a) >= len(b) else _merge(b, a)


def _build(blocks, npat):
    """Build + compile the per-core SPMD program."""
    nc = bacc.Bacc("TRN2", target_bir_lowering=False, debug=False)
    xT = nc.dram_tensor("xT", (NJ, P, NKT, CH), BF16, kind="ExternalInput")
    wqT = nc.dram_tensor("wqT", (P, NKT, C), BF16, kind="ExternalInput")
    wkT = nc.dram_tensor("wkT", (P, NKT, C), BF16, kind="ExternalInput")
    wvT = nc.dram_tensor("wvT", (P, NKT, C), BF16, kind="ExternalInput")
    woT = nc.dram_tensor("woT", (P, HPG, D), BF16, kind="ExternalInput")
    cosP = nc.dram_tensor("cosP", (P, S), BF16, kind="ExternalInput")
    sinSw = nc.dram_tensor("sinSw", (P, S), BF16, kind="ExternalInput")
    patsD = nc.dram_tensor("patsD", (P, npat, P), BF16, kind="ExternalInput")
    outD = nc.dram_tensor("out", (S, D), BF16, kind="ExternalOutput")

    QK4 = NKT // 4  # k-tiles per DMA chunk

    with tile.TileContext(nc) as tc:
        with tc.tile_pool(name="const", bufs=1) as const, \
             tc.tile_pool(name="xcp", bufs=2) as xcp, \
             tc.tile_pool(name="stg", bufs=2) as stg, \
             tc.tile_pool(name="rope", bufs=3) as ropep, \
             tc.tile_pool(name="pr", bufs=4) as prp, \
             tc.tile_pool(name="dna", bufs=2) as dnap, \
             tc.tile_pool(name="rcp", bufs=6) as rcp, \
             tc.tile_pool(name="sm", bufs=2) as smp, \
             tc.tile_pool(name="og", bufs=1) as ogp, \
             tc.tile_pool(name="psA", bufs=2, space="PSUM") as psA, \
             tc.tile_pool(name="psS", bufs=2, space="PSUM") as psS, \
             tc.tile_pool(name="psT", bufs=1, space="PSUM") as psT, \
             tc.tile_pool(name="psO", bufs=1, space="PSUM") as psO, \
             tc.tile_pool(name="psD", bufs=1, space="PSUM") as psD, \
             tc.tile_pool(name="psB", bufs=1, space="PSUM") as psB:

            # ---------------- persistent SBUF tensors ----------------
            ones_h = const.tile([P, 1], F16)
            nc.vector.memset(ones_h[:], 1.0)
            ones_b = const.tile([1, P], BF16)
            nc.vector.memset(ones_b[:], 1.0)
            pats_t = const.tile([P, npat, P], BF16)
            nc.gpsimd.dma_start(pats_t[:], patsD[:])
            cos_t = const.tile([P, S], BF16)
            sin_t = const.tile([P, S], BF16)
            nc.scalar.dma_start(cos_t[:], cosP[:])
            nc.scalar.dma_start(sin_t[:], sinSw[:])

            wq_t = const.tile([P, NKT, C], BF16)
            wk_t = const.tile([P, NKT, C], BF16)
            wv_t = const.tile([P, NKT, C], BF16)
            wo_t = const.tile([P, HPG, D], BF16)
            q_sb = const.tile([P, HPG, S], BF16)
            k_sb = const.tile([P, HPG, S], BF16)
            v_sb = const.tile([P, HPG, S // P, HD], BF16)
            attn_t = const.tile([P, HPG, S], BF16)

            xcs = [None] * NJ

            def prefetch_x(jq):
                xc = xcp.tile([P, NKT, CH], BF16, tag="xc")
                nc.sync.dma_start(xc[:], xT[jq])
                xcs[jq] = xc

            # initial loads: wq + x0 interleaved on the sync ring so the
            # first matmul group starts after ~1MB; wk/wv on the scalar ring
            # exp table preload so the first B block doesn't pay it
            warm = const.tile([1, 1], F32)
            nc.scalar.activation(warm[:], warm[:], EXP)
            # all large loads on ONE HWDGE ring in consumption order, so
            # arrival order matches the PE's consumption order at full HBM BW
            xc0 = xcp.tile([P, NKT, CH], BF16, tag="xc")
            nc.sync.dma_start(wq_t[:, 0:QK4, :], wqT[:, 0:QK4, :])
            nc.scalar.dma_start(xc0[:, 0:QK4, :], xT[0, :, 0:QK4, :])
            nc.sync.dma_start(wq_t[:, QK4:NKT, :], wqT[:, QK4:NKT, :])
            nc.scalar.dma_start(xc0[:, QK4:NKT, :], xT[0, :, QK4:NKT, :])
            xcs[0] = xc0
            nc.sync.dma_start(wk_t[:], wkT[:])
            nc.scalar.dma_start(wv_t[:], wvT[:])

            # ---------------- emitters ----------------
            def a_units(jq):
                """QKV projection for chunk jq, split into 4-matmul units."""
                units = []
                xc = lambda: xcs[jq]
                sl = slice(jq * CH, (jq + 1) * CH)
                state = {}

                def qk_unit(wt, dst, ct, su):
                    def emit():
                        if su == 0:
                            state['ps'] = psA.tile([P, CH], F32, tag="qk", name="ps_qk")
                        ps = state['ps']
                        for k in range(4 * su, 4 * su + 4):
                            nc.tensor.matmul(
                                ps[:], wt[:, k, ct * P:(ct + 1) * P],
                                xc()[:, k, :],
                                start=(k == 0), stop=(k == NKT - 1),
                                skip_group_check=True)
                        if su == 3:
                            # RoPE: out_top = x0*cos - x1*sin (sin rows 0:64
                            # hold +sin, 64:128 hold -sin, pre-swizzled)
                            pc = stg.tile([P, CH], BF16, tag="pc")
                            nc.scalar.activation(pc[:], ps[:], COPY)
                            t1 = ropep.tile([P, CH], BF16, tag="t1")
                            nc.vector.tensor_tensor(t1[:], pc[:], cos_t[:, sl], MULT)
                            t2 = ropep.tile([P, CH], BF16, tag="t2")
                            nc.vector.tensor_tensor(
                                t2[0:64, :], pc[64:128, :], sin_t[64:128, sl], MULT)
                            nc.vector.tensor_tensor(
                                t2[64:128, :], pc[0:64, :], sin_t[0:64, sl], MULT)
                            nc.vector.tensor_tensor(dst[:, ct, sl], t1[:], t2[:], ADD)
                    return emit

                def v_unit(st2, su):
                    def emit():
                        if su == 0:
                            state['psv'] = psA.tile([P, HPG, HD], F32, tag="qk", name="ps_v")
                        psv = state['psv']
                        for k in range(4 * su, 4 * su + 4):
                            nc.tensor.matmul(
                                psv[:], xc()[:, k, st2 * P:(st2 + 1) * P],
                                wv_t[:, k, :],
                                start=(k == 0), stop=(k == NKT - 1),
                                skip_group_check=True)
                        if su == 3:
                            nc.vector.tensor_copy(
                                v_sb[:, :, jq * SUB + st2, :], psv[:])
                    return emit

                for ct in range(HPG):
                    for su in range(4):
                        units.append(qk_unit(wq_t, q_sb, ct, su))
                for ct in range(HPG):
                    for su in range(4):
                        units.append(qk_unit(wk_t, k_sb, ct, su))
                for st2 in range(SUB):
                    for su in range(4):
                        units.append(v_unit(st2, su))
                return units

            rcs = {}

            def b_units(jq):
                """Attention blocks for all heads at chunk jq."""
                units = []
                qsl = slice(jq * CH, (jq + 1) * CH)
                lst = blocks[jq]
                nb = len(lst)
                for h in range(HPG):
                    state = {}

                    def blk_unit(h, n, i, c0, w, bpats):
                        def emit():
                            if n == 0:
                                state['at'] = psT.tile([P, CH], F32, tag="at", name="at_ps")
                                state['dn'] = dnap.tile([P, CH], F16, tag="dn", name="dn_acc")
                                nc.vector.memset(state['dn'][:], 0.0)
                            csl = slice(c0 * P, (c0 + w) * P)
                            qcols = slice(jq * CH + c0 * P, jq * CH + (c0 + w) * P)
                            sc = psS.tile([P, CH], F32, tag="sc")
                            nc.tensor.matmul(
                                sc[:, :w * P], k_sb[:, h, i * P:(i + 1) * P],
                                q_sb[:, h, qcols], start=True, stop=True,
                                skip_group_check=True)
                            pr = prp.tile([P, CH], BF16, tag="pr")
                            nc.scalar.activation(
                                pr[:, :w * P], sc[:, :w * P], EXP, scale=SCALE)
                            for (c_rel, pidx) in bpats:
                                nc.vector.tensor_tensor(
                                    pr[:, c_rel * P:(c_rel + 1) * P],
                                    pr[:, c_rel * P:(c_rel + 1) * P],
                                    pats_t[:, pidx, :], MULT)
                            nc.vector.tensor_tensor(
                                state['dn'][:, csl], state['dn'][:, csl],
                                pr[:, :w * P], ADD)
                            nc.tensor.matmul(
                                state['at'][:, csl], v_sb[:, h, i, :],
                                pr[:, :w * P], start=(n == 0), stop=(n == nb - 1),
                                skip_group_check=True)
                            if n == nb - 1:
                                # denominator: one ones-matmul per (h, jq);
                                # 1/d computed as exp(-ln d) on ACT (Ln, Exp
                                # and Copy share one activation table) -- no
                                # reshape DMAs, no DVE reciprocal
                                dn_ps = psD.tile([1, CH], F32, tag="sm",
                                                 name="dn_ps")
                                nc.tensor.matmul(
                                    dn_ps[:], ones_h[:], state['dn'][:],
                                    start=True, stop=True,
                                    skip_group_check=True)
                                nc.vector.tensor_copy(attn_t[:, h, qsl],
                                                      state['at'][:])
                                ln_sb = smp.tile([1, CH], F32, tag="lnsb")
                                nc.scalar.activation(
                                    ln_sb[:], dn_ps[:],
                                    mybir.ActivationFunctionType.Ln)
                                rc = rcp.tile([1, CH], BF16, tag="rc")
                                nc.scalar.activation(rc[:], ln_sb[:], EXP,
                                                     scale=-1.0)
                                rcs[(h, jq)] = rc
                        return emit

                    for n, (i, c0, w, bpats) in enumerate(lst):
                        units.append(blk_unit(h, n, i, c0, w, bpats))
                    if h > 0:
                        units.append(norm_unit(h - 1, jq))
                units.append(norm_unit(HPG - 1, jq))
                return units

            def norm_unit(h, jq):
                """Broadcast 1/denom and normalize attn rows of (h, jq)."""
                qsl = slice(jq * CH, (jq + 1) * CH)

                def emit():
                    rc = rcs.pop((h, jq))
                    bc_ps = psB.tile([P, CH], F32, tag="bc")
                    nc.tensor.matmul(bc_ps[:], ones_b[:], rc[:],
                                     start=True, stop=True,
                                     skip_group_check=True)
                    bc_sb = smp.tile([P, CH], BF16, tag="bcs")
                    nc.vector.tensor_copy(bc_sb[:], bc_ps[:])
                    nc.vector.tensor_tensor(
                        attn_t[:, h, qsl], attn_t[:, h, qsl], bc_sb[:], MULT)
                return emit

            def c_units(jq):
                """Output projection for rows [jq*512, (jq+1)*512)."""
                units = []
                for st2 in range(SUB):
                    st = jq * SUB + st2
                    state = {}

                    def unit(st, st2, dch):
                        def emit():
                            if dch == 0:
                                state['og'] = ogp.tile([P, D], BF16, tag="og", name="og")
                            parity = (st2 * (D // CH) + dch) % 2
                            pool = psO if parity == 0 else psA
                            po = pool.tile([P, CH], F32, tag="po" if parity == 0 else "qk",
                                           name="po")
                            for ct in range(HPG):
                                nc.tensor.matmul(
                                    po[:], attn_t[:, ct, st * P:(st + 1) * P],
                                    wo_t[:, ct, dch * CH:(dch + 1) * CH],
                                    start=(ct == 0), stop=(ct == HPG - 1),
                                    skip_group_check=True)
                            eng = nc.vector if parity == 0 else nc.scalar
                            if parity == 0:
                                eng.tensor_copy(
                                    state['og'][:, dch * CH:(dch + 1) * CH], po[:])
                            else:
                                eng.activation(
                                    state['og'][:, dch * CH:(dch + 1) * CH], po[:], COPY)
                            if dch == D // CH - 1:
                                nc.sync.dma_start(
                                    outD[st * P:(st + 1) * P, :], state['og'][:])
                        return emit

                    for dch in range(D // CH):
                        units.append(unit(st, st2, dch))
                return units

            # ---------------- schedule ----------------
            # t=0: A(0); t=1..3: B(t-1) + {A(t), norms(t-2), C(t-2)};
            # t=4: B(3) + {norms(2), C(2)}; t=5: norms(3), C(3)
            # t=0: A(0); t=1: B(0)+A(1); t=2: B(1)+A(2);
            # t=3: B(2)+{A(3), C(0)}; t=4: B(3)+{C(1), C(2)}; t=5: C(3)
            # (normalization rides inside the B streams with a 1-head lag)
            c_of = {3: [0], 4: [1, 2], 5: [3]}
            for t in range(6):
                fillers = []
                if t < NJ - 1:
                    fillers.append(lambda jq=t + 1: prefetch_x(jq))
                if t == 0:
                    fillers.append(lambda: nc.sync.dma_start(wo_t[:], woT[:]))
                av = a_units(t) if 1 <= t <= 3 else []
                cv = []
                for jq in c_of.get(t, []):
                    cv.extend(c_units(jq))
                fillers.extend(_rr(av, cv))
                if t == 0:
                    units = fillers + a_units(0)
                elif 1 <= t <= 4:
                    units = _merge(b_units(t - 1), fillers)
                else:
                    units = fillers
                for u in units:
                    u()

    nc.compile()
    return nc


def _prep_host(inputs):
    """Shard + transpose + bf16-convert the full inputs into 8 per-core maps."""
    x = np.asarray(inputs["x"], np.float32)
    wq = np.asarray(inputs["wq"], np.float32)
    wk = np.asarray(inputs["wk"], np.float32)
    wv = np.asarray(inputs["wv"], np.float32)
    wo = np.asarray(inputs["wo"], np.float32)
    cos = np.asarray(inputs["cos"], np.float32)
    sin = np.asarray(inputs["sin"], np.float32)
    mask = np.asarray(inputs["mask"], np.float32)
    start_p = int(inputs["start_p"])

    s = x.shape[1]
    cos_u = cos[start_p:start_p + s]          # (s, HD/2)
    sin_u = sin[start_p:start_p + s]

    # rotate-half channel permutation within each head: [evens, odds]
    perm = np.concatenate(
        [h * HD + np.concatenate([np.arange(0, HD, 2), np.arange(1, HD, 2)])
         for h in range(H)])

    cosP = np.concatenate([cos_u.T, cos_u.T], axis=0).astype(_BF16)
    sinSw = np.concatenate([sin_u.T, -sin_u.T], axis=0).astype(_BF16)

    blocks, pats = _classify_mask(mask)
    patsD = np.ascontiguousarray(pats.transpose(1, 0, 2)).astype(_BF16)

    in_maps = []
    for b in range(B):
        xb = x[b].astype(_BF16)  # (s, d)
        xTp = np.ascontiguousarray(
            xb.reshape(NJ, CH, NKT, P).transpose(0, 3, 2, 1))
        for g in range(GROUPS):
            rows = perm[g * C:(g + 1) * C]
            in_maps.append({
                "xT": xTp,
                "wqT": _pre_w(wq[rows, :].T).astype(_BF16),
                "wkT": _pre_w(wk[rows, :].T).astype(_BF16),
                "wvT": _pre_w(wv[g * C:(g + 1) * C, :].T).astype(_BF16),
                "woT": _pre_w(wo[:, g * C:(g + 1) * C].T).astype(_BF16),
                "cosP": cosP,
                "sinSw": sinSw,
                "patsD": patsD,
            })
    return in_maps, blocks, pats


def _run(inputs, trace=False):
    in_maps, blocks, pats = _prep_host(inputs)
    key = _blocks_key(blocks, pats.shape[0])
    if key not in _PROGRAM_CACHE:
        _PROGRAM_CACHE[key] = _build(blocks, pats.shape[0])
    nc = _PROGRAM_CACHE[key]
    res = bass_utils.run_bass_kernel_spmd(
        nc, in_maps, core_ids=list(range(NCORES)), trace=trace)
    out = np.zeros((B, S, D), np.float32)
    for b in range(B):
        acc = res.results[b * GROUPS]["out"].astype(np.float32)
        for g in range(1, GROUPS):
            acc = acc + res.results[b * GROUPS + g]["out"].astype(np.float32)
        out[b] = acc
    return out, res


def kernel(**inputs):
    out, _ = _run(inputs, trace=False)
    return out


# revision 12
# speedup vs baseline: 1.2260x; 1.2260x over previous
"""Trainium2 Bass kernel for nn_Attention_51307679318359.

Multi-head attention (B=2, S=2048, D=2048, H=16, HD=128) with RoPE and an
additive mask, sharded over 8 NeuronCores as (batch x head-group): each core
computes 1 batch and 4 heads (512 channels), producing a partial output that
the host sums over head-groups.

v2 design (bf16 end-to-end, fully SBUF-resident, software-pipelined):
  - All matmul operands in bf16 (PSUM accumulation stays f32): halves DMA
    and SBUF, and LDWEIGHTS runs with FWL (fp32 weight loads can't).
  - q/k/v for all 4 heads stay SBUF-resident between projection and
    attention -- no DRAM spill round trip.
  - Work is organized per sq-chunk jq (512 columns): A(jq) = QKV projection
    for chunk jq, B(jq) = attention blocks for all heads at chunk jq,
    C(jq) = output projection rows for chunk jq.  Emission interleaves the
    ACT-bound B(jq-1) blocks with PE-dense fillers (A(jq) groups, C(jq-2)
    groups) so the tensor engine never waits on the Exp chain.
  - Softmax denominator: probs blocks accumulate on DVE into an fp16 tile
    (4x DVE mode); the PE only runs one ones-vector matmul per (h, jq)
    instead of one per block.
  - Causal structure: fully-masked 128x128 sub-blocks are skipped by
    column-slicing each (i, jq) block; only mixed sub-blocks get a
    multiplicative exp(mask) pattern (a single [128,128] triangle for the
    causal mask).
"""

import math

import numpy as np

import concourse.bass as bass
import concourse.mybir as mybir
import concourse.tile as tile
from concourse import bacc
from concourse import bass_utils

try:
    import ml_dtypes
    _BF16 = ml_dtypes.bfloat16
    _FP16 = np.float16
except Exception:  # pragma: no cover
    _BF16 = None
    _FP16 = np.float16

F32 = mybir.dt.float32
BF16 = mybir.dt.bfloat16
F16 = mybir.dt.float16
ADD = mybir.AluOpType.add
MULT = mybir.AluOpType.mult
EXP = mybir.ActivationFunctionType.Exp
COPY = mybir.ActivationFunctionType.Copy

B, S, D = 2, 2048, 2048
H, HD = 16, 128
NCORES = 8
GROUPS = NCORES // B          # 4 head-groups
HPG = H // GROUPS             # 4 heads per group
C = HPG * HD                  # 512 per-core channels
P = 128
CH = 512                      # s-chunk width (both projection and attention)
NJ = S // CH                  # 4 chunks
NKT = D // P                  # 16 contraction tiles
SCALE = 1.0 / math.sqrt(HD)
NEG_THRESH = -1e8             # "masked out" threshold
SUB = CH // P                 # 4 sub-columns of 128 per chunk

_PROGRAM_CACHE = {}


def _pre_w(wT):
    """(d, c) row-major -> (128, d//128, c) partition-major contiguous."""
    d, c = wT.shape
    return np.ascontiguousarray(wT.reshape(d // P, P, c).transpose(1, 0, 2))


def _classify_mask(mask):
    """Classify the mask at 128x128 sub-block granularity.

    Returns (blocks, pats):
      blocks[jq] = list of (i, c0, w, [(c_rel, pat_idx), ...]) -- live
        sk-tile i contributes columns [c0*128, (c0+w)*128) of chunk jq;
        listed sub-columns get a multiplicative exp(mask) pattern.
      pats: (npat, 128, 128) f32 of exp(mask sub-block).
    The first block of each jq list covers the full live column range so
    the PSUM accumulation group can start with a full-width write.
    """
    maskT = np.ascontiguousarray(mask.T)
    n_i = mask.shape[0] // P
    patterns = []
    pat_idx = {}

    def add_pattern(blk):
        key = blk.tobytes()
        if key not in pat_idx:
            pat_idx[key] = len(patterns)
            with np.errstate(over='ignore'):
                patterns.append(np.exp(blk.astype(np.float64)).astype(np.float32))
        return pat_idx[key]

    def classify(jq, i, dense):
        sub_cls = []
        for c in range(SUB):
            blk = maskT[i * P:(i + 1) * P,
                        jq * CH + c * P: jq * CH + (c + 1) * P]
            if not dense and np.all(blk <= NEG_THRESH):
                sub_cls.append(('masked', None))
            elif np.all(blk == 0.0):
                sub_cls.append(('clear', None))
            else:
                sub_cls.append(('pat', blk))
        live = [c for c, (k, _) in enumerate(sub_cls) if k != 'masked']
        if not live:
            return None
        c0, c1 = live[0], live[-1]
        pats = []
        for c in range(c0, c1 + 1):
            kind, blk = sub_cls[c]
            if kind == 'masked':
                # non-contiguous live range: zero out via an all-zero pattern
                blk = np.full((P, P), -np.inf, np.float32)
                kind = 'pat'
            if kind == 'pat':
                pats.append((c - c0, add_pattern(blk)))
        return (i, c0, c1 - c0 + 1, pats)

    def build(dense):
        blocks = {}
        ok = True
        for jq in range(NJ):
            lst = []
            for i in range(n_i):
                r = classify(jq, i, dense)
                if r is not None:
                    lst.append(r)
            # first block must cover the full live column range of the chunk
            if not lst:
                ok = False
                break
            lo = min(r[1] for r in lst)
            hi = max(r[1] + r[2] for r in lst)
            full = [k for k, r in enumerate(lst) if r[1] == lo and r[1] + r[2] == hi]
            if not full:
                ok = False
                break
            lst.insert(0, lst.pop(full[0]))
            # every live column must be covered (else softmax denom is 0)
            cov = np.zeros(SUB, bool)
            for (_, c0, w, _) in lst:
                cov[c0:c0 + w] = True
            if not cov[lo:hi].all():
                ok = False
                break
            blocks[jq] = lst
        return blocks if ok else None

    blocks = build(dense=False)
    if blocks is None:
        patterns.clear()
        pat_idx.clear()
        blocks = build(dense=True)
        assert blocks is not None
    pats = np.stack(patterns, 0) if patterns else np.zeros((1, P, P), np.float32)
    return blocks, pats


def _blocks_key(blocks, npat):
    return (npat, tuple(
        (jq, tuple((i, c0, w, tuple(ps)) for (i, c0, w, ps) in lst))
        for jq, lst in sorted(blocks.items())))


def _merge(primary, fillers):
    """Interleave fillers evenly between primary units (fillers lead)."""
    out = []
    i = j = 0
    np_, nf = len(primary), len(fillers)
    while i < np_ or j < nf:
        if j < nf and (i >= np_ or j * np_ <= i * nf):
            out.append(fillers[j]); j += 1
        else:
            out.append(primary[i]); i += 1
    return out


def _rr(a, b):
    """Round-robin two lists proportionally."""
    return _merge(a, b) if len(a) >= len(b) else _merge(b, a)


def _build(blocks, npat):
    """Build + compile the per-core SPMD program."""
    nc = bacc.Bacc("TRN2", target_bir_lowering=False, debug=False)
    xT = nc.dram_tensor("xT", (NJ, P, NKT, CH), BF16, kind="ExternalInput")
    wqT = nc.dram_tensor("wqT", (P, NKT, C), BF16, kind="ExternalInput")
    wkT = nc.dram_tensor("wkT", (P, NKT, C), BF16, kind="ExternalInput")
    wvT = nc.dram_tensor("wvT", (P, NKT, C), BF16, kind="ExternalInput")
    woT = nc.dram_tensor("woT", (P, HPG, D), BF16, kind="ExternalInput")
    cosP = nc.dram_tensor("cosP", (P, S), BF16, kind="ExternalInput")
    sinSw = nc.dram_tensor("sinSw", (P, S), BF16, kind="ExternalInput")
    patsD = nc.dram_tensor("patsD", (P, npat, P), BF16, kind="ExternalInput")
    outD = nc.dram_tensor("out", (S, D), BF16, kind="ExternalOutput")

    QK4 = NKT // 4  # k-tiles per DMA chunk

    with tile.TileContext(nc) as tc:
        with tc.tile_pool(name="const", bufs=1) as const, \
             tc.tile_pool(name="xcp", bufs=2) as xcp, \
             tc.tile_pool(name="stg", bufs=2) as stg, \
             tc.tile_pool(name="rope", bufs=3) as ropep, \
             tc.tile_pool(name="pr", bufs=4) as prp, \
             tc.tile_pool(name="dna", bufs=2) as dnap, \
             tc.tile_pool(name="rcp", bufs=6) as rcp, \
             tc.tile_pool(name="sm", bufs=2) as smp, \
             tc.tile_pool(name="og", bufs=1) as ogp, \
             tc.tile_pool(name="psA", bufs=2, space="PSUM") as psA, \
             tc.tile_pool(name="psS", bufs=2, space="PSUM") as psS, \
             tc.tile_pool(name="psT", bufs=1, space="PSUM") as psT, \
             tc.tile_pool(name="psO", bufs=1, space="PSUM") as psO, \
             tc.tile_pool(name="psD", bufs=1, space="PSUM") as psD, \
             tc.tile_pool(name="psB", bufs=1, space="PSUM") as psB:

            # ---------------- persistent SBUF tensors ----------------
            ones_h = const.tile([P, 1], F16)
            nc.vector.memset(ones_h[:], 1.0)
            ones_b = const.tile([1, P], BF16)
            nc.vector.memset(ones_b[:], 1.0)
            pats_t = const.tile([P, npat, P], BF16)
            nc.gpsimd.dma_start(pats_t[:], patsD[:])
            cos_t = const.tile([P, S], BF16)
            sin_t = const.tile([P, S], BF16)
            nc.scalar.dma_start(cos_t[:], cosP[:])
            nc.scalar.dma_start(sin_t[:], sinSw[:])

            wq_t = const.tile([P, NKT, C], BF16)
            wk_t = const.tile([P, NKT, C], BF16)
            wv_t = const.tile([P, NKT, C], BF16)
            wo_t = const.tile([P, HPG, D], BF16)
            q_sb = const.tile([P, HPG, S], BF16)
            k_sb = const.tile([P, HPG, S], BF16)
            v_sb = const.tile([P, HPG, S // P, HD], BF16)
            attn_t = const.tile([P, HPG, S], BF16)

            xcs = [None] * NJ

            def prefetch_x(jq):
                xc = xcp.tile([P, NKT, CH], BF16, tag="xc")
                nc.sync.dma_start(xc[:], xT[jq])
                xcs[jq] = xc

            # initial loads: wq + x0 interleaved on the sync ring so the
            # first matmul group starts after ~1MB; wk/wv on the scalar ring
            # exp table preload so the first B block doesn't pay it
            warm = const.tile([1, 1], F32)
            nc.scalar.activation(warm[:], warm[:], EXP)
            # all large loads on ONE HWDGE ring in consumption order, so
            # arrival order matches the PE's consumption order at full HBM BW
            xc0 = xcp.tile([P, NKT, CH], BF16, tag="xc")
            nc.sync.dma_start(wq_t[:, 0:QK4, :], wqT[:, 0:QK4, :])
            nc.scalar.dma_start(xc0[:, 0:QK4, :], xT[0, :, 0:QK4, :])
            nc.sync.dma_start(wq_t[:, QK4:NKT, :], wqT[:, QK4:NKT, :])
            nc.scalar.dma_start(xc0[:, QK4:NKT, :], xT[0, :, QK4:NKT, :])
            xcs[0] = xc0
            nc.sync.dma_start(wk_t[:], wkT[:])
            nc.scalar.dma_start(wv_t[:], wvT[:])

            # ---------------- emitters ----------------
            def a_units(jq):
                """QKV projection for chunk jq, split into 4-matmul units."""
                units = []
                xc = lambda: xcs[jq]
                sl = slice(jq * CH, (jq + 1) * CH)
                state = {}

                def qk_unit(wt, dst, ct, su):
                    def emit():
                        if su == 0:
                            state['ps'] = psA.tile([P, CH], F32, tag="qk", name="ps_qk")
                        ps = state['ps']
                        for k in range(4 * su, 4 * su + 4):
                            nc.tensor.matmul(
                                ps[:], wt[:, k, ct * P:(ct + 1) * P],
                                xc()[:, k, :],
                                start=(k == 0), stop=(k == NKT - 1),
                                skip_group_check=True)
                        if su == 3:
                            # RoPE: out_top = x0*cos - x1*sin (sin rows 0:64
                            # hold +sin, 64:128 hold -sin, pre-swizzled)
                            pc = stg.tile([P, CH], BF16, tag="pc")
                            nc.scalar.activation(pc[:], ps[:], COPY)
                            t1 = ropep.tile([P, CH], BF16, tag="t1")
                            nc.vector.tensor_tensor(t1[:], pc[:], cos_t[:, sl], MULT)
                            t2 = ropep.tile([P, CH], BF16, tag="t2")
                            nc.vector.tensor_tensor(
                                t2[0:64, :], pc[64:128, :], sin_t[64:128, sl], MULT)
                            nc.vector.tensor_tensor(
                                t2[64:128, :], pc[0:64, :], sin_t[0:64, sl], MULT)
                            nc.vector.tensor_tensor(dst[:, ct, sl], t1[:], t2[:], ADD)
                    return emit

                def v_unit(st2, su):
                    def emit():
                        if su == 0:
                            state['psv'] = psA.tile([P, HPG, HD], F32, tag="qk", name="ps_v")
                        psv = state['psv']
                        for k in range(4 * su, 4 * su + 4):
                            nc.tensor.matmul(
                                psv[:], xc()[:, k, st2 * P:(st2 + 1) * P],
                                wv_t[:, k, :],
                                start=(k == 0), stop=(k == NKT - 1),
                                skip_group_check=True)
                        if su == 3:
                            nc.vector.tensor_copy(
                                v_sb[:, :, jq * SUB + st2, :], psv[:])
                    return emit

                for ct in range(HPG):
                    for su in range(4):
                        units.append(qk_unit(wq_t, q_sb, ct, su))
                for ct in range(HPG):
                    for su in range(4):
                        units.append(qk_unit(wk_t, k_sb, ct, su))
                for st2 in range(SUB):
                    for su in range(4):
                        units.append(v_unit(st2, su))
                return units

            rcs = {}

            def b_units(jq):
                """Attention blocks for all heads at chunk jq."""
                units = []
                qsl = slice(jq * CH, (jq + 1) * CH)
                lst = blocks[jq]
                nb = len(lst)
                for h in range(HPG):
                    state = {}

                    def blk_unit(h, n, i, c0, w, bpats):
                        def emit():
                            if n == 0:
                                state['at'] = psT.tile([P, CH], F32, tag="at", name="at_ps")
                                state['dn'] = dnap.tile([P, CH], F16, tag="dn", name="dn_acc")
                                nc.vector.memset(state['dn'][:], 0.0)
                            csl = slice(c0 * P, (c0 + w) * P)
                            qcols = slice(jq * CH + c0 * P, jq * CH + (c0 + w) * P)
                            sc = psS.tile([P, CH], F32, tag="sc")
                            nc.tensor.matmul(
                                sc[:, :w * P], k_sb[:, h, i * P:(i + 1) * P],
                                q_sb[:, h, qcols], start=True, stop=True,
                                skip_group_check=True)
                            pr = prp.tile([P, CH], BF16, tag="pr")
                            nc.scalar.activation(
                                pr[:, :w * P], sc[:, :w * P], EXP, scale=SCALE)
                            for (c_rel, pidx) in bpats:
                                nc.vector.tensor_tensor(
                                    pr[:, c_rel * P:(c_rel + 1) * P],
                                    pr[:, c_rel * P:(c_rel + 1) * P],
                                    pats_t[:, pidx, :], MULT)
                            nc.vector.tensor_tensor(
                                state['dn'][:, csl], state['dn'][:, csl],
                                pr[:, :w * P], ADD)
                            nc.tensor.matmul(
                                state['at'][:, csl], v_sb[:, h, i, :],
                                pr[:, :w * P], start=(n == 0), stop=(n == nb - 1),
                                skip_group_check=True)
                            if n == nb - 1:
                                # denominator: one ones-matmul per (h, jq)
                                dn_ps = psD.tile([1, CH], F32, tag="sm",
                                                 name="dn_ps")
                                nc.tensor.matmul(
                                    dn_ps[:], ones_h[:], state['dn'][:],
                                    start=True, stop=True,
                                    skip_group_check=True)
                                nc.vector.tensor_copy(attn_t[:, h, qsl],
                                                      state['at'][:])
                                # reciprocal cost scales with free-size per
                                # lane: fold (1,512)->(4,128) via HWDGE DMA
                                dn_sb = smp.tile([1, CH], F32, tag="dnsb")
                                nc.vector.tensor_copy(dn_sb[:], dn_ps[:])
                                dn4 = smp.tile([SUB, P], F32, tag="dn4")
                                nc.sync.dma_start(dn4[:], dn_sb[:])
                                rc4 = smp.tile([SUB, P], F32, tag="rc4")
                                nc.vector.reciprocal(rc4[:], dn4[:])
                                rc4b = smp.tile([SUB, P], BF16, tag="rc4b")
                                nc.vector.tensor_copy(rc4b[:], rc4[:])
                                rc = rcp.tile([1, CH], BF16, tag="rc")
                                nc.sync.dma_start(rc[:], rc4b[:])
                                rcs[(h, jq)] = rc
                        return emit

                    for n, (i, c0, w, bpats) in enumerate(lst):
                        units.append(blk_unit(h, n, i, c0, w, bpats))
                    if h > 0:
                        units.append(norm_unit(h - 1, jq))
                units.append(norm_unit(HPG - 1, jq))
                return units

            def norm_unit(h, jq):
                """Broadcast 1/denom and normalize attn rows of (h, jq)."""
                qsl = slice(jq * CH, (jq + 1) * CH)

                def emit():
                    rc = rcs.pop((h, jq))
                    bc_ps = psB.tile([P, CH], F32, tag="bc")
                    nc.tensor.matmul(bc_ps[:], ones_b[:], rc[:],
                                     start=True, stop=True,
                                     skip_group_check=True)
                    bc_sb = smp.tile([P, CH], BF16, tag="bcs")
                    nc.vector.tensor_copy(bc_sb[:], bc_ps[:])
                    nc.vector.tensor_tensor(
                        attn_t[:, h, qsl], attn_t[:, h, qsl], bc_sb[:], MULT)
                return emit

            def c_units(jq):
                """Output projection for rows [jq*512, (jq+1)*512)."""
                units = []
                for st2 in range(SUB):
                    st = jq * SUB + st2
                    state = {}

                    def unit(st, st2, dch):
                        def emit():
                            if dch == 0:
                                state['og'] = ogp.tile([P, D], BF16, tag="og", name="og")
                            parity = (st2 * (D // CH) + dch) % 2
                            pool = psO if parity == 0 else psA
                            po = pool.tile([P, CH], F32, tag="po" if parity == 0 else "qk",
                                           name="po")
                            for ct in range(HPG):
                                nc.tensor.matmul(
                                    po[:], attn_t[:, ct, st * P:(st + 1) * P],
                                    wo_t[:, ct, dch * CH:(dch + 1) * CH],
                                    start=(ct == 0), stop=(ct == HPG - 1),
                                    skip_group_check=True)
                            eng = nc.vector if parity == 0 else nc.scalar
                            if parity == 0:
                                eng.tensor_copy(
                                    state['og'][:, dch * CH:(dch + 1) * CH], po[:])
                            else:
                                eng.activation(
                                    state['og'][:, dch * CH:(dch + 1) * CH], po[:], COPY)
                            if dch == D // CH - 1:
                                nc.sync.dma_start(
                                    outD[st * P:(st + 1) * P, :], state['og'][:])
                        return emit

                    for dch in range(D // CH):
                        units.append(unit(st, st2, dch))
                return units

            # ---------------- schedule ----------------
            # t=0: A(0); t=1..3: B(t-1) + {A(t), norms(t-2), C(t-2)};
            # t=4: B(3) + {norms(2), C(2)}; t=5: norms(3), C(3)
            # t=0: A(0); t=1: B(0)+A(1); t=2: B(1)+A(2);
            # t=3: B(2)+{A(3), C(0)}; t=4: B(3)+{C(1), C(2)}; t=5: C(3)
            # (normalization rides inside the B streams with a 1-head lag)
            c_of = {3: [0], 4: [1, 2], 5: [3]}
            for t in range(6):
                fillers = []
                if t < NJ - 1:
                    fillers.append(lambda jq=t + 1: prefetch_x(jq))
                if t == 0:
                    fillers.append(lambda: nc.sync.dma_start(wo_t[:], woT[:]))
                av = a_units(t) if 1 <= t <= 3 else []
                cv = []
                for jq in c_of.get(t, []):
                    cv.extend(c_units(jq))
                fillers.extend(_rr(av, cv))
                if t == 0:
                    units = fillers + a_units(0)
                elif 1 <= t <= 4:
                    units = _merge(b_units(t - 1), fillers)
                else:
                    units = fillers
                for u in units:
                    u()

    nc.compile()
    return nc


def _prep_host(inputs):
    """Shard + transpose + bf16-convert the full inputs into 8 per-core maps."""
    x = np.asarray(inputs["x"], np.float32)
    wq = np.asarray(inputs["wq"], np.float32)
    wk = np.asarray(inputs["wk"], np.float32)
    wv = np.asarray(inputs["wv"], np.float32)
    wo = np.asarray(inputs["wo"], np.float32)
    cos = np.asarray(inputs["cos"], np.float32)
    sin = np.asarray(inputs["sin"], np.float32)
    mask = np.asarray(inputs["mask"], np.float32)
    start_p = int(inputs["start_p"])

    s = x.shape[1]
    cos_u = cos[start_p:start_p + s]          # (s, HD/2)
    sin_u = sin[start_p:start_p + s]

    # rotate-half channel permutation within each head: [evens, odds]
    perm = np.concatenate(
        [h * HD + np.concatenate([np.arange(0, HD, 2), np.arange(1, HD, 2)])
         for h in range(H)])

    cosP = np.concatenate([cos_u.T, cos_u.T], axis=0).astype(_BF16)
    sinSw = np.concatenate([sin_u.T, -sin_u.T], axis=0).astype(_BF16)

    blocks, pats = _classify_mask(mask)
    patsD = np.ascontiguousarray(pats.transpose(1, 0, 2)).astype(_BF16)

    in_maps = []
    for b in range(B):
        xb = x[b].astype(_BF16)  # (s, d)
        xTp = np.ascontiguousarray(
            xb.reshape(NJ, CH, NKT, P).transpose(0, 3, 2, 1))
        for g in range(GROUPS):
            rows = perm[g * C:(g + 1) * C]
            in_maps.append({
                "xT": xTp,
                "wqT": _pre_w(wq[rows, :].T).astype(_BF16),
                "wkT": _pre_w(wk[rows, :].T).astype(_BF16),
                "wvT": _pre_w(wv[g * C:(g + 1) * C, :].T).astype(_BF16),
                "woT": _pre_w(wo[:, g * C:(g + 1) * C].T).astype(_BF16),
                "cosP": cosP,
                "sinSw": sinSw,
                "patsD": patsD,
            })
    return in_maps, blocks, pats


def _run(inputs, trace=False):
    in_maps, blocks, pats = _prep_host(inputs)
    key = _blocks_key(blocks, pats.shape[0])
    if key not in _PROGRAM_CACHE:
        _PROGRAM_CACHE[key] = _build(blocks, pats.shape[0])
    nc = _PROGRAM_CACHE[key]
    res = bass_utils.run_bass_kernel_spmd(
        nc, in_maps, core_ids=list(range(NCORES)), trace=trace)
    out = np.zeros((B, S, D), np.float32)
    for b in range(B):
        acc = res.results[b * GROUPS]["out"].astype(np.float32)
        for g in range(1, GROUPS):
            acc = acc + res.results[b * GROUPS + g]["out"].astype(np.float32)
        out[b] = acc
    return out, res


def kernel(**inputs):
    out, _ = _run(inputs, trace=False)
    return out


# revision 13
# speedup vs baseline: 1.2795x; 1.0437x over previous
"""Trainium2 Bass kernel for nn_Attention_51307679318359.

Multi-head attention (B=2, S=2048, D=2048, H=16, HD=128) with RoPE and an
additive mask, sharded over 8 NeuronCores as (batch x head-group): each core
computes 1 batch and 4 heads (512 channels), producing a partial output that
the host sums over head-groups.

v2 design (bf16 end-to-end, fully SBUF-resident, software-pipelined):
  - All matmul operands in bf16 (PSUM accumulation stays f32): halves DMA
    and SBUF, and LDWEIGHTS runs with FWL (fp32 weight loads can't).
  - q/k/v for all 4 heads stay SBUF-resident between projection and
    attention -- no DRAM spill round trip.
  - Work is organized per sq-chunk jq (512 columns): A(jq) = QKV projection
    for chunk jq, B(jq) = attention blocks for all heads at chunk jq,
    C(jq) = output projection rows for chunk jq.  Emission interleaves the
    ACT-bound B(jq-1) blocks with PE-dense fillers (A(jq) groups, C(jq-2)
    groups) so the tensor engine never waits on the Exp chain.
  - Softmax denominator: probs blocks accumulate on DVE into an fp16 tile
    (4x DVE mode); the PE only runs one ones-vector matmul per (h, jq)
    instead of one per block.
  - Causal structure: fully-masked 128x128 sub-blocks are skipped by
    column-slicing each (i, jq) block; only mixed sub-blocks get a
    multiplicative exp(mask) pattern (a single [128,128] triangle for the
    causal mask).
"""

import math

import numpy as np

import concourse.bass as bass
import concourse.mybir as mybir
import concourse.tile as tile
from concourse import bacc
from concourse import bass_utils

try:
    import ml_dtypes
    _BF16 = ml_dtypes.bfloat16
    _FP16 = np.float16
except Exception:  # pragma: no cover
    _BF16 = None
    _FP16 = np.float16

F32 = mybir.dt.float32
BF16 = mybir.dt.bfloat16
F16 = mybir.dt.float16
ADD = mybir.AluOpType.add
MULT = mybir.AluOpType.mult
EXP = mybir.ActivationFunctionType.Exp
COPY = mybir.ActivationFunctionType.Copy

B, S, D = 2, 2048, 2048
H, HD = 16, 128
NCORES = 8
GROUPS = NCORES // B          # 4 head-groups
HPG = H // GROUPS             # 4 heads per group
C = HPG * HD                  # 512 per-core channels
P = 128
CH = 512                      # s-chunk width (both projection and attention)
NJ = S // CH                  # 4 chunks
NKT = D // P                  # 16 contraction tiles
SCALE = 1.0 / math.sqrt(HD)
NEG_THRESH = -1e8             # "masked out" threshold
SUB = CH // P                 # 4 sub-columns of 128 per chunk

_PROGRAM_CACHE = {}


def _pre_w(wT):
    """(d, c) row-major -> (128, d//128, c) partition-major contiguous."""
    d, c = wT.shape
    return np.ascontiguousarray(wT.reshape(d // P, P, c).transpose(1, 0, 2))


def _classify_mask(mask):
    """Classify the mask at 128x128 sub-block granularity.

    Returns (blocks, pats):
      blocks[jq] = list of (i, c0, w, [(c_rel, pat_idx), ...]) -- live
        sk-tile i contributes columns [c0*128, (c0+w)*128) of chunk jq;
        listed sub-columns get a multiplicative exp(mask) pattern.
      pats: (npat, 128, 128) f32 of exp(mask sub-block).
    The first block of each jq list covers the full live column range so
    the PSUM accumulation group can start with a full-width write.
    """
    maskT = np.ascontiguousarray(mask.T)
    n_i = mask.shape[0] // P
    patterns = []
    pat_idx = {}

    def add_pattern(blk):
        key = blk.tobytes()
        if key not in pat_idx:
            pat_idx[key] = len(patterns)
            with np.errstate(over='ignore'):
                patterns.append(np.exp(blk.astype(np.float64)).astype(np.float32))
        return pat_idx[key]

    def classify(jq, i, dense):
        sub_cls = []
        for c in range(SUB):
            blk = maskT[i * P:(i + 1) * P,
                        jq * CH + c * P: jq * CH + (c + 1) * P]
            if not dense and np.all(blk <= NEG_THRESH):
                sub_cls.append(('masked', None))
            elif np.all(blk == 0.0):
                sub_cls.append(('clear', None))
            else:
                sub_cls.append(('pat', blk))
        live = [c for c, (k, _) in enumerate(sub_cls) if k != 'masked']
        if not live:
            return None
        c0, c1 = live[0], live[-1]
        pats = []
        for c in range(c0, c1 + 1):
            kind, blk = sub_cls[c]
            if kind == 'masked':
                # non-contiguous live range: zero out via an all-zero pattern
                blk = np.full((P, P), -np.inf, np.float32)
                kind = 'pat'
            if kind == 'pat':
                pats.append((c - c0, add_pattern(blk)))
        return (i, c0, c1 - c0 + 1, pats)

    def build(dense):
        blocks = {}
        ok = True
        for jq in range(NJ):
            lst = []
            for i in range(n_i):
                r = classify(jq, i, dense)
                if r is not None:
                    lst.append(r)
            # first block must cover the full live column range of the chunk
            if not lst:
                ok = False
                break
            lo = min(r[1] for r in lst)
            hi = max(r[1] + r[2] for r in lst)
            full = [k for k, r in enumerate(lst) if r[1] == lo and r[1] + r[2] == hi]
            if not full:
                ok = False
                break
            lst.insert(0, lst.pop(full[0]))
            # every live column must be covered (else softmax denom is 0)
            cov = np.zeros(SUB, bool)
            for (_, c0, w, _) in lst:
                cov[c0:c0 + w] = True
            if not cov[lo:hi].all():
                ok = False
                break
            blocks[jq] = lst
        return blocks if ok else None

    blocks = build(dense=False)
    if blocks is None:
        patterns.clear()
        pat_idx.clear()
        blocks = build(dense=True)
        assert blocks is not None
    pats = np.stack(patterns, 0) if patterns else np.zeros((1, P, P), np.float32)
    return blocks, pats


def _blocks_key(blocks, npat):
    return (npat, tuple(
        (jq, tuple((i, c0, w, tuple(ps)) for (i, c0, w, ps) in lst))
        for jq, lst in sorted(blocks.items())))


def _merge(primary, fillers):
    """Interleave fillers evenly between primary units (fillers lead)."""
    out = []
    i = j = 0
    np_, nf = len(primary), len(fillers)
    while i < np_ or j < nf:
        if j < nf and (i >= np_ or j * np_ <= i * nf):
            out.append(fillers[j]); j += 1
        else:
            out.append(primary[i]); i += 1
    return out


def _rr(a, b):
    """Round-robin two lists proportionally."""
    return _merge(a, b) if len(a) >= len(b) else _merge(b, a)


def _build(blocks, npat):
    """Build + compile the per-core SPMD program."""
    nc = bacc.Bacc("TRN2", target_bir_lowering=False, debug=False)
    xT = nc.dram_tensor("xT", (NJ, P, NKT, CH), BF16, kind="ExternalInput")
    wqT = nc.dram_tensor("wqT", (P, NKT, C), BF16, kind="ExternalInput")
    wkT = nc.dram_tensor("wkT", (P, NKT, C), BF16, kind="ExternalInput")
    wvT = nc.dram_tensor("wvT", (P, NKT, C), BF16, kind="ExternalInput")
    woT = nc.dram_tensor("woT", (P, HPG, D), BF16, kind="ExternalInput")
    cosP = nc.dram_tensor("cosP", (P, S), BF16, kind="ExternalInput")
    sinSw = nc.dram_tensor("sinSw", (P, S), BF16, kind="ExternalInput")
    patsD = nc.dram_tensor("patsD", (P, npat, P), BF16, kind="ExternalInput")
    outD = nc.dram_tensor("out", (S, D), BF16, kind="ExternalOutput")

    QK4 = NKT // 4  # k-tiles per DMA chunk

    with tile.TileContext(nc) as tc:
        with tc.tile_pool(name="const", bufs=1) as const, \
             tc.tile_pool(name="xcp", bufs=2) as xcp, \
             tc.tile_pool(name="stg", bufs=2) as stg, \
             tc.tile_pool(name="rope", bufs=3) as ropep, \
             tc.tile_pool(name="pr", bufs=4) as prp, \
             tc.tile_pool(name="dna", bufs=2) as dnap, \
             tc.tile_pool(name="rcp", bufs=6) as rcp, \
             tc.tile_pool(name="sm", bufs=2) as smp, \
             tc.tile_pool(name="og", bufs=1) as ogp, \
             tc.tile_pool(name="psA", bufs=2, space="PSUM") as psA, \
             tc.tile_pool(name="psS", bufs=2, space="PSUM") as psS, \
             tc.tile_pool(name="psT", bufs=1, space="PSUM") as psT, \
             tc.tile_pool(name="psO", bufs=1, space="PSUM") as psO, \
             tc.tile_pool(name="psD", bufs=1, space="PSUM") as psD, \
             tc.tile_pool(name="psB", bufs=1, space="PSUM") as psB:

            # ---------------- persistent SBUF tensors ----------------
            ones_h = const.tile([P, 1], F16)
            nc.vector.memset(ones_h[:], 1.0)
            ones_b = const.tile([1, P], BF16)
            nc.vector.memset(ones_b[:], 1.0)
            pats_t = const.tile([P, npat, P], BF16)
            nc.gpsimd.dma_start(pats_t[:], patsD[:])
            cos_t = const.tile([P, S], BF16)
            sin_t = const.tile([P, S], BF16)
            nc.scalar.dma_start(cos_t[:], cosP[:])
            nc.scalar.dma_start(sin_t[:], sinSw[:])

            wq_t = const.tile([P, NKT, C], BF16)
            wk_t = const.tile([P, NKT, C], BF16)
            wv_t = const.tile([P, NKT, C], BF16)
            wo_t = const.tile([P, HPG, D], BF16)
            q_sb = const.tile([P, HPG, S], BF16)
            k_sb = const.tile([P, HPG, S], BF16)
            v_sb = const.tile([P, HPG, S // P, HD], BF16)
            attn_t = const.tile([P, HPG, S], BF16)

            xcs = [None] * NJ

            def prefetch_x(jq):
                xc = xcp.tile([P, NKT, CH], BF16, tag="xc")
                nc.sync.dma_start(xc[:], xT[jq])
                xcs[jq] = xc

            # initial loads: wq + x0 interleaved on the sync ring so the
            # first matmul group starts after ~1MB; wk/wv on the scalar ring
            # exp table preload so the first B block doesn't pay it
            warm = const.tile([1, 1], F32)
            nc.scalar.activation(warm[:], warm[:], EXP)
            # all large loads on ONE HWDGE ring in consumption order, so
            # arrival order matches the PE's consumption order at full HBM BW
            xc0 = xcp.tile([P, NKT, CH], BF16, tag="xc")
            for q4 in range(4):
                nc.sync.dma_start(wq_t[:, q4 * QK4:(q4 + 1) * QK4, :],
                                  wqT[:, q4 * QK4:(q4 + 1) * QK4, :])
                nc.scalar.dma_start(xc0[:, q4 * QK4:(q4 + 1) * QK4, :],
                                    xT[0, :, q4 * QK4:(q4 + 1) * QK4, :])
            xcs[0] = xc0
            for q4 in range(4):
                eng = nc.sync if q4 < 2 else nc.scalar
                eng.dma_start(wk_t[:, q4 * QK4:(q4 + 1) * QK4, :],
                              wkT[:, q4 * QK4:(q4 + 1) * QK4, :])
            for q4 in range(4):
                eng = nc.sync if q4 < 2 else nc.scalar
                eng.dma_start(wv_t[:, q4 * QK4:(q4 + 1) * QK4, :],
                              wvT[:, q4 * QK4:(q4 + 1) * QK4, :])

            # ---------------- emitters ----------------
            def a_units(jq):
                """QKV projection for chunk jq, split into 4-matmul units."""
                units = []
                xc = lambda: xcs[jq]
                sl = slice(jq * CH, (jq + 1) * CH)
                state = {}

                def qk_unit(wt, dst, ct, su):
                    def emit():
                        if su == 0:
                            state['ps'] = psA.tile([P, CH], F32, tag="qk", name="ps_qk")
                        ps = state['ps']
                        for k in range(4 * su, 4 * su + 4):
                            nc.tensor.matmul(
                                ps[:], wt[:, k, ct * P:(ct + 1) * P],
                                xc()[:, k, :],
                                start=(k == 0), stop=(k == NKT - 1),
                                skip_group_check=True)
                        if su == 3:
                            # RoPE: out_top = x0*cos - x1*sin (sin rows 0:64
                            # hold +sin, 64:128 hold -sin, pre-swizzled)
                            pc = stg.tile([P, CH], BF16, tag="pc")
                            nc.scalar.activation(pc[:], ps[:], COPY)
                            t1 = ropep.tile([P, CH], BF16, tag="t1")
                            nc.vector.tensor_tensor(t1[:], pc[:], cos_t[:, sl], MULT)
                            t2 = ropep.tile([P, CH], BF16, tag="t2")
                            nc.vector.tensor_tensor(
                                t2[0:64, :], pc[64:128, :], sin_t[64:128, sl], MULT)
                            nc.vector.tensor_tensor(
                                t2[64:128, :], pc[0:64, :], sin_t[0:64, sl], MULT)
                            nc.vector.tensor_tensor(dst[:, ct, sl], t1[:], t2[:], ADD)
                    return emit

                def v_unit(st2, su):
                    def emit():
                        if su == 0:
                            state['psv'] = psA.tile([P, HPG, HD], F32, tag="qk", name="ps_v")
                        psv = state['psv']
                        for k in range(4 * su, 4 * su + 4):
                            nc.tensor.matmul(
                                psv[:], xc()[:, k, st2 * P:(st2 + 1) * P],
                                wv_t[:, k, :],
                                start=(k == 0), stop=(k == NKT - 1),
                                skip_group_check=True)
                        if su == 3:
                            nc.vector.tensor_copy(
                                v_sb[:, :, jq * SUB + st2, :], psv[:])
                    return emit

                for ct in range(HPG):
                    for su in range(4):
                        units.append(qk_unit(wq_t, q_sb, ct, su))
                for ct in range(HPG):
                    for su in range(4):
                        units.append(qk_unit(wk_t, k_sb, ct, su))
                for st2 in range(SUB):
                    for su in range(4):
                        units.append(v_unit(st2, su))
                return units

            rcs = {}

            def b_units(jq):
                """Attention blocks for all heads at chunk jq."""
                units = []
                qsl = slice(jq * CH, (jq + 1) * CH)
                lst = blocks[jq]
                nb = len(lst)
                for h in range(HPG):
                    state = {}

                    def blk_unit(h, n, i, c0, w, bpats):
                        def emit():
                            if n == 0:
                                state['at'] = psT.tile([P, CH], F32, tag="at", name="at_ps")
                                state['dn'] = dnap.tile([P, CH], F16, tag="dn", name="dn_acc")
                                nc.vector.memset(state['dn'][:], 0.0)
                            csl = slice(c0 * P, (c0 + w) * P)
                            qcols = slice(jq * CH + c0 * P, jq * CH + (c0 + w) * P)
                            sc = psS.tile([P, CH], F32, tag="sc")
                            nc.tensor.matmul(
                                sc[:, :w * P], k_sb[:, h, i * P:(i + 1) * P],
                                q_sb[:, h, qcols], start=True, stop=True,
                                skip_group_check=True)
                            pr = prp.tile([P, CH], BF16, tag="pr")
                            nc.scalar.activation(
                                pr[:, :w * P], sc[:, :w * P], EXP, scale=SCALE)
                            for (c_rel, pidx) in bpats:
                                nc.vector.tensor_tensor(
                                    pr[:, c_rel * P:(c_rel + 1) * P],
                                    pr[:, c_rel * P:(c_rel + 1) * P],
                                    pats_t[:, pidx, :], MULT)
                            nc.vector.tensor_tensor(
                                state['dn'][:, csl], state['dn'][:, csl],
                                pr[:, :w * P], ADD)
                            nc.tensor.matmul(
                                state['at'][:, csl], v_sb[:, h, i, :],
                                pr[:, :w * P], start=(n == 0), stop=(n == nb - 1),
                                skip_group_check=True)
                            if n == nb - 1:
                                # denominator: one ones-matmul per (h, jq)
                                dn_ps = psD.tile([1, CH], F32, tag="sm",
                                                 name="dn_ps")
                                nc.tensor.matmul(
                                    dn_ps[:], ones_h[:], state['dn'][:],
                                    start=True, stop=True,
                                    skip_group_check=True)
                                nc.vector.tensor_copy(attn_t[:, h, qsl],
                                                      state['at'][:])
                                # reciprocal cost scales with free-size per
                                # lane: fold (1,512)->(4,128) via HWDGE DMA
                                dn_sb = smp.tile([1, CH], F32, tag="dnsb")
                                nc.vector.tensor_copy(dn_sb[:], dn_ps[:])
                                dn4 = smp.tile([SUB, P], F32, tag="dn4")
                                nc.gpsimd.dma_start(dn4[:], dn_sb[:])
                                rc4 = smp.tile([SUB, P], F32, tag="rc4")
                                nc.vector.reciprocal(rc4[:], dn4[:])
                                rc4b = smp.tile([SUB, P], BF16, tag="rc4b")
                                nc.vector.tensor_copy(rc4b[:], rc4[:])
                                rc = rcp.tile([1, CH], BF16, tag="rc")
                                nc.gpsimd.dma_start(rc[:], rc4b[:])
                                rcs[(h, jq)] = rc
                        return emit

                    for n, (i, c0, w, bpats) in enumerate(lst):
                        units.append(blk_unit(h, n, i, c0, w, bpats))
                    if h > 0:
                        units.append(norm_unit(h - 1, jq))
                units.append(norm_unit(HPG - 1, jq))
                return units

            def norm_unit(h, jq):
                """Broadcast 1/denom and normalize attn rows of (h, jq)."""
                qsl = slice(jq * CH, (jq + 1) * CH)

                def emit():
                    rc = rcs.pop((h, jq))
                    bc_ps = psB.tile([P, CH], F32, tag="bc")
                    nc.tensor.matmul(bc_ps[:], ones_b[:], rc[:],
                                     start=True, stop=True,
                                     skip_group_check=True)
                    bc_sb = smp.tile([P, CH], BF16, tag="bcs")
                    nc.vector.tensor_copy(bc_sb[:], bc_ps[:])
                    nc.vector.tensor_tensor(
                        attn_t[:, h, qsl], attn_t[:, h, qsl], bc_sb[:], MULT)
                return emit

            def c_units(jq):
                """Output projection for rows [jq*512, (jq+1)*512)."""
                units = []
                for st2 in range(SUB):
                    st = jq * SUB + st2
                    state = {}

                    def unit(st, st2, dch):
                        def emit():
                            if dch == 0:
                                state['og'] = ogp.tile([P, D], BF16, tag="og", name="og")
                            parity = (st2 * (D // CH) + dch) % 2
                            pool = psO if parity == 0 else psA
                            po = pool.tile([P, CH], F32, tag="po" if parity == 0 else "qk",
                                           name="po")
                            for ct in range(HPG):
                                nc.tensor.matmul(
                                    po[:], attn_t[:, ct, st * P:(st + 1) * P],
                                    wo_t[:, ct, dch * CH:(dch + 1) * CH],
                                    start=(ct == 0), stop=(ct == HPG - 1),
                                    skip_group_check=True)
                            eng = nc.vector if parity == 0 else nc.scalar
                            if parity == 0:
                                eng.tensor_copy(
                                    state['og'][:, dch * CH:(dch + 1) * CH], po[:])
                            else:
                                eng.activation(
                                    state['og'][:, dch * CH:(dch + 1) * CH], po[:], COPY)
                            if dch == D // CH - 1:
                                nc.sync.dma_start(
                                    outD[st * P:(st + 1) * P, :], state['og'][:])
                        return emit

                    for dch in range(D // CH):
                        units.append(unit(st, st2, dch))
                return units

            # ---------------- schedule ----------------
            # t=0: A(0); t=1..3: B(t-1) + {A(t), norms(t-2), C(t-2)};
            # t=4: B(3) + {norms(2), C(2)}; t=5: norms(3), C(3)
            # t=0: A(0); t=1: B(0)+A(1); t=2: B(1)+A(2);
            # t=3: B(2)+{A(3), C(0)}; t=4: B(3)+{C(1), C(2)}; t=5: C(3)
            # (normalization rides inside the B streams with a 1-head lag)
            c_of = {3: [0], 4: [1, 2], 5: [3]}
            for t in range(6):
                fillers = []
                if t < NJ - 1:
                    fillers.append(lambda jq=t + 1: prefetch_x(jq))
                if t == 0:
                    fillers.append(lambda: nc.sync.dma_start(wo_t[:], woT[:]))
                av = a_units(t) if 1 <= t <= 3 else []
                cv = []
                for jq in c_of.get(t, []):
                    cv.extend(c_units(jq))
                fillers.extend(_rr(av, cv))
                if t == 0:
                    units = fillers + a_units(0)
                elif 1 <= t <= 4:
                    units = _merge(b_units(t - 1), fillers)
                else:
                    units = fillers
                for u in units:
                    u()

    nc.compile()
    return nc


def _prep_host(inputs):
    """Shard + transpose + bf16-convert the full inputs into 8 per-core maps."""
    x = np.asarray(inputs["x"], np.float32)
    wq = np.asarray(inputs["wq"], np.float32)
    wk = np.asarray(inputs["wk"], np.float32)
    wv = np.asarray(inputs["wv"], np.float32)
    wo = np.asarray(inputs["wo"], np.float32)
    cos = np.asarray(inputs["cos"], np.float32)
    sin = np.asarray(inputs["sin"], np.float32)
    mask = np.asarray(inputs["mask"], np.float32)
    start_p = int(inputs["start_p"])

    s = x.shape[1]
    cos_u = cos[start_p:start_p + s]          # (s, HD/2)
    sin_u = sin[start_p:start_p + s]

    # rotate-half channel permutation within each head: [evens, odds]
    perm = np.concatenate(
        [h * HD + np.concatenate([np.arange(0, HD, 2), np.arange(1, HD, 2)])
         for h in range(H)])

    cosP = np.concatenate([cos_u.T, cos_u.T], axis=0).astype(_BF16)
    sinSw = np.concatenate([sin_u.T, -sin_u.T], axis=0).astype(_BF16)

    blocks, pats = _classify_mask(mask)
    patsD = np.ascontiguousarray(pats.transpose(1, 0, 2)).astype(_BF16)

    in_maps = []
    for b in range(B):
        xb = x[b].astype(_BF16)  # (s, d)
        xTp = np.ascontiguousarray(
            xb.reshape(NJ, CH, NKT, P).transpose(0, 3, 2, 1))
        for g in range(GROUPS):
            rows = perm[g * C:(g + 1) * C]
            in_maps.append({
                "xT": xTp,
                "wqT": _pre_w(wq[rows, :].T).astype(_BF16),
                "wkT": _pre_w(wk[rows, :].T).astype(_BF16),
                "wvT": _pre_w(wv[g * C:(g + 1) * C, :].T).astype(_BF16),
                "woT": _pre_w(wo[:, g * C:(g + 1) * C].T).astype(_BF16),
                "cosP": cosP,
                "sinSw": sinSw,
                "patsD": patsD,
            })
    return in_maps, blocks, pats


def _run(inputs, trace=False):
    in_maps, blocks, pats = _prep_host(inputs)
    key = _blocks_key(blocks, pats.shape[0])
    if key not in _PROGRAM_CACHE:
        _PROGRAM_CACHE[key] = _build(blocks, pats.shape[0])
    nc = _PROGRAM_CACHE[key]
    res = bass_utils.run_bass_kernel_spmd(
        nc, in_maps, core_ids=list(range(NCORES)), trace=trace)
    out = np.zeros((B, S, D), np.float32)
    for b in range(B):
        acc = res.results[b * GROUPS]["out"].astype(np.float32)
        for g in range(1, GROUPS):
            acc = acc + res.results[b * GROUPS + g]["out"].astype(np.float32)
        out[b] = acc
    return out, res


def kernel(**inputs):
    out, _ = _run(inputs, trace=False)
    return out
